# revision 29
# baseline (speedup 1.0000x reference)
"""ColumnParallelLinearWithLoRA kernel for 8 Trainium2 NeuronCores.

Computes out = x @ W.T + bias + 2.0 * lora, where lora routes each token
through one of 8 LoRA adapters (rank 16): lora[s] = B[idx_s] @ (A[idx_s] @ x[s]).

Sharding: data-parallel over tokens (1024 tokens per core). Each core keeps
its x-shard transposed and resident in SBUF, streams the full transposed
weight from HBM exactly once, and computes the LoRA path only for its own
tokens. W/bias/A/B are replicated; the host gathers and transposes the 8
per-core [d_out, tokens] shards back into [S, d_out].

Matmuls run in float32r (fp32 with 11-bit mantissa, 4x the fp32 rate on the
PE): operands are pre-rounded on the host; the LoRA A-projections are rounded
by the DVE when the adapter one-hot mask (built on host) is applied.

The startup x-load (16.8 MB) is overlapped with compute: the LoRA-A
projections (2 PSUM banks) and 3/4 of the first weight block (6 banks) run
in a ki-loop paced by x arrival; the deferred quarter (mc=3) runs as a small
second pass (mb0c) re-fetching its 2 MB of W. DMA queues are split: x/m1/
bias/btf + even-ki W on the Sync HWDGE queue, packed-A + mb0/mb0c/odd-ki W
on the Scalar HWDGE queue, output evictions on GpSimd SWDGE.
"""
from contextlib import ExitStack

import numpy as np

import concourse.bacc as bacc
import concourse.tile as tile
from concourse import mybir
from concourse.bass_utils import run_bass_kernel_spmd

S, D_IN, D_OUT, R, L = 8192, 4096, 4096, 16, 8
SCALING = 2.0
N_CORES = 8
P = 128
S_SH = S // N_CORES          # 1024 tokens per core
LR = L * R                   # 128 stacked adapter rows
KC = D_IN // P               # 32 contraction chunks
TC = S_SH // 512             # 2 token chunks of 512
MB = D_OUT // 512            # 8 output blocks of 512
MC = 4                       # 128-row output chunks per block

f32 = mybir.dt.float32
f32r = mybir.dt.float32r

_CACHE = {}


def round_f32r(a: np.ndarray) -> np.ndarray:
    """Round-to-nearest to fp32r (11 explicit mantissa bits, low 12 bits 0)."""
    u = np.ascontiguousarray(a, dtype=np.float32).view(np.uint32)
    rbit = (u >> np.uint32(12)) & np.uint32(1)
    ur = (u + np.uint32(0x7FF) + rbit) & np.uint32(0xFFFFF000)
    return ur.view(np.float32)


def _build_nc():
    nc = bacc.Bacc("TRN2", target_bir_lowering=False, debug=False,
                   num_devices=N_CORES)
    # All big operands are host-packed into PE-ready, per-partition-contiguous
    # layouts so each DMA descriptor covers an 8-16 KiB run (the DMA rings
    # are descriptor-rate-bound at ~2 KiB runs).
    # xt[p, ki*S_SH + n]      = x[n, ki*P + p]        (shard, transposed)
    # wt[p, (mb*KC+ki)*512+n] = weight[mb*512+n, ki*P+p]
    xt = nc.dram_tensor("xt", [P, KC * S_SH], f32r, kind="ExternalInput").ap()
    wt = nc.dram_tensor("wt", [P, MB * KC * 512], f32r,
                        kind="ExternalInput").ap()
    wt3 = nc.dram_tensor("wt3", [P, KC * P], f32r, kind="ExternalInput").ap()
    at = nc.dram_tensor("at", [P, KC * LR], f32r, kind="ExternalInput").ap()
    btf = nc.dram_tensor("btf", [LR, D_OUT], f32r, kind="ExternalInput").ap()
    m1 = nc.dram_tensor("m1", [LR, S_SH], f32, kind="ExternalInput").ap()
    bias = nc.dram_tensor("bias", [D_OUT], f32, kind="ExternalInput").ap()
    ot = nc.dram_tensor("ot", [D_OUT, S_SH], f32, kind="ExternalOutput").ap()

    ot_r = ot.rearrange("(j p) n -> j p n", p=P)

    with tile.TileContext(nc) as tc, ExitStack() as ctx:
        xpool = ctx.enter_context(tc.tile_pool(name="xres", bufs=KC // 2))
        cpool = ctx.enter_context(tc.tile_pool(name="const", bufs=1))
        apool = ctx.enter_context(tc.tile_pool(name="abar", bufs=TC))
        wpool = ctx.enter_context(tc.tile_pool(name="wstream", bufs=5))
        w0pool = ctx.enter_context(tc.tile_pool(name="w0c", bufs=2))
        opool = ctx.enter_context(tc.tile_pool(name="oevict", bufs=2))
        psum = ctx.enter_context(tc.tile_pool(name="ps", bufs=8, space="PSUM"))

        # ---- Residents (loads interleaved into the startup loop below) --
        xres = []

        def xs(ki, t):  # rhs slice for contraction chunk ki, token chunk t
            kp, j = divmod(ki, 2)
            off = j * S_SH + t * 512
            return xres[kp][:, off:off + 512]

        at_sb = cpool.tile([P, KC * LR], f32r, tag="at")
        m1_sb = cpool.tile([P, S_SH], f32, tag="m1")
        bias_sb = cpool.tile([P, D_OUT // P], f32, tag="bias")
        btf_sb = cpool.tile([P, D_OUT], f32r, tag="btf")

        def finalize(po_mc_t, mb, mc, split_dma=False):
            """LoRA B-side accumulate + bias eviction + output DMA."""
            j = mb * MC + mc
            osb = opool.tile([P, S_SH], f32, tag="oevict", name=f"o{mb}_{mc}")
            for t in range(TC):
                nc.tensor.matmul(po_mc_t[t][:], btf_sb[:, j * P:(j + 1) * P],
                                 abar[t][:], start=False, stop=True)
                nc.scalar.activation(osb[:, t * 512:(t + 1) * 512],
                                     po_mc_t[t][:],
                                     mybir.ActivationFunctionType.Identity,
                                     bias=bias_sb[:, j:j + 1])
                if split_dma:
                    nc.gpsimd.dma_start(
                        ot_r[j][:, t * 512:(t + 1) * 512],
                        osb[:, t * 512:(t + 1) * 512])
            if not split_dma:
                nc.gpsimd.dma_start(ot_r[j][:], osb[:])

        W_G = 2  # ki steps per W fetch: 512 KiB DMAs, 8 KiB/descriptor

        def w_chunk(mb, g, eng, bufname):
            wtile = wpool.tile([P, W_G * 512], f32r, tag="wstream",
                               name=bufname)
            base = (mb * KC + W_G * g) * 512
            eng.dma_start(wtile[:], wt[:, base:base + W_G * 512])
            return wtile

        def w_slice(wtile, ki, mc):
            off = (ki % W_G) * 512 + mc * P
            return wtile[:, off:off + P]

        # ---- Startup ki loop: LoRA-A (2 banks) + mb0 mc0-2 (6 banks) ---
        # All startup DMAs ride ONE queue (sync) in exact consumption order:
        # the two HWDGE rings alternate in multi-MB slugs when driven
        # concurrently, costing ~20% aggregate bandwidth.
        pa = [psum.tile([P, 512], f32, tag="ps", name=f"pa{t}")
              for t in range(TC)]
        po0 = [[psum.tile([P, 512], f32, tag="ps", name=f"po0_{mc}_{t}")
                for t in range(TC)] for mc in range(3)]
        for kp in range(KC // 2):
            if kp % 4 == 0:  # at chunk g covers ki 8g..8g+7 = kp 4g..4g+3
                g = kp // 4
                sl = slice(g * KC * LR // 4, (g + 1) * KC * LR // 4)
                nc.sync.dma_start(at_sb[:, sl], at[:, sl])
            xk = xpool.tile([P, 2 * S_SH], f32r, tag="xres", name=f"x{kp}")
            nc.sync.dma_start(xk[:], xt[:, kp * 2 * S_SH:(kp + 1) * 2 * S_SH])
            xres.append(xk)
            wtile = w_chunk(0, kp, nc.sync, f"w0_{kp}")
            for ki in (2 * kp, 2 * kp + 1):
                for t in range(TC):
                    nc.tensor.matmul(pa[t][:], at_sb[:, ki * P:(ki + 1) * P],
                                     xs(ki, t), start=(ki == 0),
                                     stop=(ki == KC - 1))
                for mc in range(3):
                    for t in range(TC):
                        nc.tensor.matmul(po0[mc][t][:], w_slice(wtile, ki, mc),
                                         xs(ki, t), start=(ki == 0),
                                         stop=False)
        nc.sync.dma_start(m1_sb[:], m1[:])
        nc.sync.dma_start(bias_sb[:], bias.rearrange("(j p) -> p j", p=P))
        nc.sync.dma_start(btf_sb[:], btf[:])
        abar = []
        for t in range(TC):
            ab = apool.tile([P, 512], f32r, tag="abar", name=f"ab{t}")
            nc.vector.tensor_mul(ab[:], pa[t][:],
                                 m1_sb[:, t * 512:(t + 1) * 512])
            abar.append(ab)
        # ---- mb0c: deferred mc=3 of block 0 (host-packed W re-fetch) ---
        # Runs before the mc0-2 finalizes: po3 only needs the pa slots
        # (freed by the abar muls), so the PE refills right after startup.
        po3 = [psum.tile([P, 512], f32, tag="ps", name=f"po3_{t}")
               for t in range(TC)]
        for g in range(4):  # each chunk covers 8 contraction steps
            w3 = w0pool.tile([P, 8 * P], f32r, tag="w0c", name=f"w3_{g}")
            nc.sync.dma_start(w3[:], wt3[:, g * 8 * P:(g + 1) * 8 * P])
            for kk in range(8):
                ki = g * 8 + kk
                for t in range(TC):
                    nc.tensor.matmul(po3[t][:], w3[:, kk * P:(kk + 1) * P],
                                     xs(ki, t), start=(ki == 0), stop=False)
        for mc in range(3):
            finalize(po0[mc], 0, mc)
        finalize(po3, 0, 3)

        # ---- Main blocks mb 1..7 ---------------------------------------
        for mb in range(1, MB):
            po = [[psum.tile([P, 512], f32, tag="ps", name=f"po{mb}_{mc}_{t}")
                   for t in range(TC)] for mc in range(MC)]
            for ki in range(KC):
                if ki % W_G == 0:
                    g = ki // W_G
                    eng = nc.sync if g % 2 == 0 else nc.scalar
                    wtile = w_chunk(mb, g, eng, f"w{mb}_{g}")
                for mc in range(MC):
                    for t in range(TC):
                        nc.tensor.matmul(po[mc][t][:], w_slice(wtile, ki, mc),
                                         xs(ki, t), start=(ki == 0), stop=False)
            for mc in range(MC):
                finalize(po[mc], mb, mc, split_dma=(mb == MB - 1))

    nc.compile()
    return nc


def get_nc():
    if "nc" not in _CACHE:
        _CACHE["nc"] = _build_nc()
    return _CACHE["nc"]


def prep_in_maps(x, weight, bias, A_buffer, B_buffer, weight_indices):
    """Host-side sharding + layout transforms + fp32r pre-rounding."""
    x = np.asarray(x, dtype=np.float32)
    weight = np.asarray(weight, dtype=np.float32)
    bias = np.asarray(bias, dtype=np.float32)
    A_buffer = np.asarray(A_buffer, dtype=np.float32)
    B_buffer = np.asarray(B_buffer, dtype=np.float32)
    weight_indices = np.asarray(weight_indices)

    wt_std = round_f32r(weight.T)                          # [D_IN, D_OUT]
    wt_4d = wt_std.reshape(KC, P, MB, 512)                 # [ki, p, mb, n]
    # wt[p, (mb*KC + ki)*512 + n] = weight[mb*512+n, ki*P+p]
    wt = np.ascontiguousarray(
        wt_4d.transpose(1, 2, 0, 3).reshape(P, MB * KC * 512))
    # wt3[p, ki*P + n] = wt_std[ki*P + p, 384 + n]: packed mc=3 of block 0
    wt3 = np.ascontiguousarray(
        wt_std.reshape(KC, P, D_OUT)[:, :, 384:512]
        .transpose(1, 0, 2).reshape(P, KC * P))
    # at[p, ki*LR + lr] = A_stack[lr, ki*P + p]  (PE-ready packed layout)
    a_stack_t = round_f32r(A_buffer.reshape(LR, D_IN).T)   # [D_IN, LR]
    at = np.ascontiguousarray(
        a_stack_t.reshape(KC, P, LR).transpose(1, 0, 2).reshape(P, KC * LR))
    # btf[l*R+r, m] = B_buffer[0, l, m, r] * SCALING
    btf = round_f32r(B_buffer[0].transpose(0, 2, 1).reshape(LR, D_OUT)
                     * SCALING)
    m1_full = np.repeat(
        (np.arange(L)[:, None] == weight_indices[None, :]), R, axis=0
    ).astype(np.float32)                                   # [LR, S]
    xt_full = round_f32r(x).T                              # view [D_IN, S]

    in_maps = []
    for c in range(N_CORES):
        sl = slice(c * S_SH, (c + 1) * S_SH)
        # xt[p, ki*S_SH + n] = x[c*S_SH + n, ki*P + p]
        xt_c = np.ascontiguousarray(
            xt_full[:, sl].reshape(KC, P, S_SH)
            .transpose(1, 0, 2).reshape(P, KC * S_SH))
        in_maps.append({
            "xt": xt_c,
            "wt": wt,
            "wt3": wt3,
            "at": at,
            "btf": btf,
            "m1": np.ascontiguousarray(m1_full[:, sl]),
            "bias": bias,
        })
    return in_maps


def gather(results):
    out = np.empty((S, D_OUT), dtype=np.float32)
    for c in range(N_CORES):
        out[c * S_SH:(c + 1) * S_SH, :] = results[c]["ot"].T
    return out


def kernel(x, weight, bias, A_buffer, B_buffer, weight_indices):
    nc = get_nc()
    in_maps = prep_in_maps(x, weight, bias, A_buffer, B_buffer, weight_indices)
    res = run_bass_kernel_spmd(nc, in_maps, list(range(N_CORES)))
    return gather(res.results)


# revision 30
# speedup vs baseline: 1.0142x; 1.0142x over previous
"""ColumnParallelLinearWithLoRA kernel for 8 Trainium2 NeuronCores.

Computes out = x @ W.T + bias + 2.0 * lora, where lora routes each token
through one of 8 LoRA adapters (rank 16): lora[s] = B[idx_s] @ (A[idx_s] @ x[s]).

Sharding: data-parallel over tokens (1024 tokens per core). Each core keeps
its x-shard transposed and resident in SBUF, streams the full transposed
weight from HBM exactly once, and computes the LoRA path only for its own
tokens. W/bias/A/B are replicated; the host gathers and transposes the 8
per-core [d_out, tokens] shards back into [S, d_out].

Matmuls run in float32r (fp32 with 11-bit mantissa, 4x the fp32 rate on the
PE): operands are pre-rounded on the host; the LoRA A-projections are rounded
by the DVE when the adapter one-hot mask (built on host) is applied.

The startup x-load (16.8 MB) is overlapped with compute: the LoRA-A
projections (2 PSUM banks) and 3/4 of the first weight block (6 banks) run
in a ki-loop paced by x arrival; the deferred quarter (mc=3) runs as a small
second pass (mb0c) re-fetching its 2 MB of W. DMA queues are split: x/m1/
bias/btf + even-ki W on the Sync HWDGE queue, packed-A + mb0/mb0c/odd-ki W
on the Scalar HWDGE queue, output evictions on GpSimd SWDGE.
"""
from contextlib import ExitStack

import numpy as np

import concourse.bacc as bacc
import concourse.tile as tile
from concourse import mybir
from concourse.bass_utils import run_bass_kernel_spmd

S, D_IN, D_OUT, R, L = 8192, 4096, 4096, 16, 8
SCALING = 2.0
N_CORES = 8
P = 128
S_SH = S // N_CORES          # 1024 tokens per core
LR = L * R                   # 128 stacked adapter rows
KC = D_IN // P               # 32 contraction chunks
TC = S_SH // 512             # 2 token chunks of 512
MB = D_OUT // 512            # 8 output blocks of 512
MC = 4                       # 128-row output chunks per block

f32 = mybir.dt.float32
f32r = mybir.dt.float32r

_CACHE = {}


def round_f32r(a: np.ndarray) -> np.ndarray:
    """Round-to-nearest to fp32r (11 explicit mantissa bits, low 12 bits 0)."""
    u = np.ascontiguousarray(a, dtype=np.float32).view(np.uint32)
    rbit = (u >> np.uint32(12)) & np.uint32(1)
    ur = (u + np.uint32(0x7FF) + rbit) & np.uint32(0xFFFFF000)
    return ur.view(np.float32)


def _build_nc():
    nc = bacc.Bacc("TRN2", target_bir_lowering=False, debug=False,
                   num_devices=N_CORES)
    xt = nc.dram_tensor("xt", [D_IN, S_SH], f32r, kind="ExternalInput").ap()
    wt = nc.dram_tensor("wt", [D_IN, D_OUT], f32r, kind="ExternalInput").ap()
    wt3 = nc.dram_tensor("wt3", [P, KC * P], f32r, kind="ExternalInput").ap()
    at = nc.dram_tensor("at", [P, KC * LR], f32r, kind="ExternalInput").ap()
    btf = nc.dram_tensor("btf", [LR, D_OUT], f32r, kind="ExternalInput").ap()
    m1 = nc.dram_tensor("m1", [LR, S_SH], f32, kind="ExternalInput").ap()
    bias = nc.dram_tensor("bias", [D_OUT], f32, kind="ExternalInput").ap()
    ot = nc.dram_tensor("ot", [D_OUT, S_SH], f32, kind="ExternalOutput").ap()

    # x ki-pair view: [kp, p, j, n] with ki = 2*kp + j
    xt_r = xt.rearrange("(kp j p) n -> kp p j n", p=P, j=2)
    wt_r = wt.rearrange("(ki p) n -> ki p n", p=P)
    ot_r = ot.rearrange("(j p) n -> j p n", p=P)

    with tile.TileContext(nc) as tc, ExitStack() as ctx:
        xpool = ctx.enter_context(tc.tile_pool(name="xres", bufs=KC // 2))
        cpool = ctx.enter_context(tc.tile_pool(name="const", bufs=1))
        apool = ctx.enter_context(tc.tile_pool(name="abar", bufs=TC))
        wpool = ctx.enter_context(tc.tile_pool(name="wstream", bufs=8))
        w0pool = ctx.enter_context(tc.tile_pool(name="w0c", bufs=2))
        opool = ctx.enter_context(tc.tile_pool(name="oevict", bufs=3))
        psum = ctx.enter_context(tc.tile_pool(name="ps", bufs=8, space="PSUM"))

        # ---- Residents -------------------------------------------------
        # x-shard as 16 ki-pair tiles (1 MB DMAs, 4 KiB/descriptor), sync q.
        xres = []
        for kp in range(KC // 2):
            xk = xpool.tile([P, 2 * S_SH], f32r, tag="xres", name=f"x{kp}")
            nc.sync.dma_start(
                xk[:].rearrange("p (j n) -> p j n", j=2), xt_r[kp])
            xres.append(xk)

        def xs(ki, t):  # rhs slice for contraction chunk ki, token chunk t
            kp, j = divmod(ki, 2)
            off = j * S_SH + t * 512
            return xres[kp][:, off:off + 512]

        # Packed A (host layout [p, ki, lr]): 4 DMAs so ki=0 arrives early.
        at_sb = cpool.tile([P, KC * LR], f32r, tag="at")
        for g in range(4):
            sl = slice(g * KC * LR // 4, (g + 1) * KC * LR // 4)
            nc.scalar.dma_start(at_sb[:, sl], at[:, sl])
        m1_sb = cpool.tile([P, S_SH], f32, tag="m1")
        nc.sync.dma_start(m1_sb[:], m1[:])
        bias_sb = cpool.tile([P, D_OUT // P], f32, tag="bias")
        nc.sync.dma_start(bias_sb[:], bias.rearrange("(j p) -> p j", p=P))
        btf_sb = cpool.tile([P, D_OUT], f32r, tag="btf")
        nc.sync.dma_start(btf_sb[:], btf[:])

        def finalize(po_mc_t, mb, mc, split_dma=False):
            """LoRA B-side accumulate + bias eviction + output DMA."""
            j = mb * MC + mc
            osb = opool.tile([P, S_SH], f32, tag="oevict", name=f"o{mb}_{mc}")
            for t in range(TC):
                nc.tensor.matmul(po_mc_t[t][:], btf_sb[:, j * P:(j + 1) * P],
                                 abar[t][:], start=False, stop=True)
                nc.scalar.activation(osb[:, t * 512:(t + 1) * 512],
                                     po_mc_t[t][:],
                                     mybir.ActivationFunctionType.Identity,
                                     bias=bias_sb[:, j:j + 1])
                if split_dma:
                    nc.gpsimd.dma_start(
                        ot_r[j][:, t * 512:(t + 1) * 512],
                        osb[:, t * 512:(t + 1) * 512])
            if not split_dma:
                nc.gpsimd.dma_start(ot_r[j][:], osb[:])

        # ---- Startup ki loop: LoRA-A (2 banks) + mb0 mc0-2 (6 banks) ---
        pa = [psum.tile([P, 512], f32, tag="ps", name=f"pa{t}")
              for t in range(TC)]
        po0 = [[psum.tile([P, 512], f32, tag="ps", name=f"po0_{mc}_{t}")
                for t in range(TC)] for mc in range(3)]
        for ki in range(KC):
            wtile = wpool.tile([P, 512], f32r, tag="wstream", name=f"w0_{ki}")
            nc.scalar.dma_start(wtile[:], wt_r[ki][:, 0:512])
            for t in range(TC):
                nc.tensor.matmul(pa[t][:], at_sb[:, ki * P:(ki + 1) * P],
                                 xs(ki, t), start=(ki == 0), stop=(ki == KC - 1))
            for mc in range(3):
                for t in range(TC):
                    nc.tensor.matmul(po0[mc][t][:],
                                     wtile[:, mc * P:(mc + 1) * P],
                                     xs(ki, t), start=(ki == 0), stop=False)
        abar = []
        for t in range(TC):
            ab = apool.tile([P, 512], f32r, tag="abar", name=f"ab{t}")
            nc.vector.tensor_mul(ab[:], pa[t][:],
                                 m1_sb[:, t * 512:(t + 1) * 512])
            abar.append(ab)
        # ---- mb0c: deferred mc=3 of block 0 (host-packed W re-fetch) ---
        # Runs before the mc0-2 finalizes: po3 only needs the pa slots
        # (freed by the abar muls), so the PE refills right after startup.
        po3 = [psum.tile([P, 512], f32, tag="ps", name=f"po3_{t}")
               for t in range(TC)]
        for g in range(4):  # each chunk covers 8 contraction steps
            w3 = w0pool.tile([P, 8 * P], f32r, tag="w0c", name=f"w3_{g}")
            nc.scalar.dma_start(w3[:], wt3[:, g * 8 * P:(g + 1) * 8 * P])
            for kk in range(8):
                ki = g * 8 + kk
                for t in range(TC):
                    nc.tensor.matmul(po3[t][:], w3[:, kk * P:(kk + 1) * P],
                                     xs(ki, t), start=(ki == 0), stop=False)
        for mc in range(3):
            finalize(po0[mc], 0, mc)
        finalize(po3, 0, 3)

        # ---- Main blocks mb 1..7 ---------------------------------------
        for mb in range(1, MB):
            po = [[psum.tile([P, 512], f32, tag="ps", name=f"po{mb}_{mc}_{t}")
                   for t in range(TC)] for mc in range(MC)]
            for ki in range(KC):
                wtile = wpool.tile([P, 512], f32r, tag="wstream",
                                   name=f"w{mb}_{ki}")
                eng = nc.sync if ki % 2 == 0 else nc.scalar
                eng.dma_start(wtile[:], wt_r[ki][:, mb * 512:(mb + 1) * 512])
                for mc in range(MC):
                    for t in range(TC):
                        nc.tensor.matmul(po[mc][t][:],
                                         wtile[:, mc * P:(mc + 1) * P],
                                         xs(ki, t), start=(ki == 0), stop=False)
            for mc in range(MC):
                finalize(po[mc], mb, mc, split_dma=(mb == MB - 1))

    nc.compile()
    return nc


def get_nc():
    if "nc" not in _CACHE:
        _CACHE["nc"] = _build_nc()
    return _CACHE["nc"]


def prep_in_maps(x, weight, bias, A_buffer, B_buffer, weight_indices):
    """Host-side sharding + layout transforms + fp32r pre-rounding."""
    x = np.asarray(x, dtype=np.float32)
    weight = np.asarray(weight, dtype=np.float32)
    bias = np.asarray(bias, dtype=np.float32)
    A_buffer = np.asarray(A_buffer, dtype=np.float32)
    B_buffer = np.asarray(B_buffer, dtype=np.float32)
    weight_indices = np.asarray(weight_indices)

    wt = round_f32r(weight.T)                              # [D_IN, D_OUT]
    # wt3[p, ki*P + n] = wt[ki*P + p, 384 + n]: packed mc=3 cols of block 0
    wt3 = np.ascontiguousarray(
        wt.reshape(KC, P, D_OUT)[:, :, 384:512]
        .transpose(1, 0, 2).reshape(P, KC * P))
    # at[p, ki*LR + lr] = A_stack[lr, ki*P + p]  (PE-ready packed layout)
    a_stack_t = round_f32r(A_buffer.reshape(LR, D_IN).T)   # [D_IN, LR]
    at = np.ascontiguousarray(
        a_stack_t.reshape(KC, P, LR).transpose(1, 0, 2).reshape(P, KC * LR))
    # btf[l*R+r, m] = B_buffer[0, l, m, r] * SCALING
    btf = round_f32r(B_buffer[0].transpose(0, 2, 1).reshape(LR, D_OUT)
                     * SCALING)
    m1_full = np.repeat(
        (np.arange(L)[:, None] == weight_indices[None, :]), R, axis=0
    ).astype(np.float32)                                   # [LR, S]
    xt_full = round_f32r(x).T                              # view [D_IN, S]

    in_maps = []
    for c in range(N_CORES):
        sl = slice(c * S_SH, (c + 1) * S_SH)
        in_maps.append({
            "xt": np.ascontiguousarray(xt_full[:, sl]),
            "wt": wt,
            "wt3": wt3,
            "at": at,
            "btf": btf,
            "m1": np.ascontiguousarray(m1_full[:, sl]),
            "bias": bias,
        })
    return in_maps


def gather(results):
    out = np.empty((S, D_OUT), dtype=np.float32)
    for c in range(N_CORES):
        out[c * S_SH:(c + 1) * S_SH, :] = results[c]["ot"].T
    return out


def kernel(x, weight, bias, A_buffer, B_buffer, weight_indices):
    nc = get_nc()
    in_maps = prep_in_maps(x, weight, bias, A_buffer, B_buffer, weight_indices)
    res = run_bass_kernel_spmd(nc, in_maps, list(range(N_CORES)))
    return gather(res.results)


# revision 33
# speedup vs baseline: 1.0197x; 1.0054x over previous
"""ColumnParallelLinearWithLoRA kernel for 8 Trainium2 NeuronCores.

Computes out = x @ W.T + bias + 2.0 * lora, where lora routes each token
through one of 8 LoRA adapters (rank 16): lora[s] = B[idx_s] @ (A[idx_s] @ x[s]).

Sharding: data-parallel over tokens (1024 tokens per core). Each core keeps
its x-shard transposed and resident in SBUF, streams the full transposed
weight from HBM exactly once, and computes the LoRA path only for its own
tokens. W/bias/A/B are replicated; the host gathers and transposes the 8
per-core [d_out, tokens] shards back into [S, d_out].

Matmuls run in float32r (fp32 with 11-bit mantissa, 4x the fp32 rate on the
PE): operands are pre-rounded on the host; the LoRA A-projections are rounded
by the DVE when the adapter one-hot mask (built on host) is applied.

The startup x-load (16.8 MB) is overlapped with compute: the LoRA-A
projections (2 PSUM banks) and 3/4 of the first weight block (6 banks) run
in a ki-loop paced by x arrival; the deferred quarter (mc=3) runs as a small
second pass (mb0c) re-fetching its 2 MB of W. DMA queues are split: x/m1/
bias/btf + even-ki W on the Sync HWDGE queue, packed-A + mb0/mb0c/odd-ki W
on the Scalar HWDGE queue, output evictions on GpSimd SWDGE.
"""
from contextlib import ExitStack

import numpy as np

import concourse.bacc as bacc
import concourse.tile as tile
from concourse import mybir
from concourse.bass_utils import run_bass_kernel_spmd

S, D_IN, D_OUT, R, L = 8192, 4096, 4096, 16, 8
SCALING = 2.0
N_CORES = 8
P = 128
S_SH = S // N_CORES          # 1024 tokens per core
LR = L * R                   # 128 stacked adapter rows
KC = D_IN // P               # 32 contraction chunks
TC = S_SH // 512             # 2 token chunks of 512
MB = D_OUT // 512            # 8 output blocks of 512
MC = 4                       # 128-row output chunks per block

f32 = mybir.dt.float32
f32r = mybir.dt.float32r

_CACHE = {}


def round_f32r(a: np.ndarray) -> np.ndarray:
    """Round-to-nearest to fp32r (11 explicit mantissa bits, low 12 bits 0)."""
    u = np.ascontiguousarray(a, dtype=np.float32).view(np.uint32)
    rbit = (u >> np.uint32(12)) & np.uint32(1)
    ur = (u + np.uint32(0x7FF) + rbit) & np.uint32(0xFFFFF000)
    return ur.view(np.float32)


def _build_nc():
    nc = bacc.Bacc("TRN2", target_bir_lowering=False, debug=False,
                   num_devices=N_CORES)
    xt = nc.dram_tensor("xt", [D_IN, S_SH], f32r, kind="ExternalInput").ap()
    wt = nc.dram_tensor("wt", [D_IN, D_OUT], f32r, kind="ExternalInput").ap()
    wt3 = nc.dram_tensor("wt3", [P, KC * P], f32r, kind="ExternalInput").ap()
    at = nc.dram_tensor("at", [P, KC * LR], f32r, kind="ExternalInput").ap()
    btf = nc.dram_tensor("btf", [LR, D_OUT], f32r, kind="ExternalInput").ap()
    m1 = nc.dram_tensor("m1", [LR, S_SH], f32, kind="ExternalInput").ap()
    bias = nc.dram_tensor("bias", [D_OUT], f32, kind="ExternalInput").ap()
    ot = nc.dram_tensor("ot", [D_OUT, S_SH], f32, kind="ExternalOutput").ap()

    # x ki-pair view: [kp, p, j, n] with ki = 2*kp + j
    xt_r = xt.rearrange("(kp j p) n -> kp p j n", p=P, j=2)
    wt_r = wt.rearrange("(ki p) n -> ki p n", p=P)
    ot_r = ot.rearrange("(j p) n -> j p n", p=P)

    with tile.TileContext(nc) as tc, ExitStack() as ctx:
        xpool = ctx.enter_context(tc.tile_pool(name="xres", bufs=KC // 2))
        cpool = ctx.enter_context(tc.tile_pool(name="const", bufs=1))
        apool = ctx.enter_context(tc.tile_pool(name="abar", bufs=TC))
        wpool = ctx.enter_context(tc.tile_pool(name="wstream", bufs=8))
        w0pool = ctx.enter_context(tc.tile_pool(name="w0c", bufs=2))
        opool = ctx.enter_context(tc.tile_pool(name="oevict", bufs=3))
        psum = ctx.enter_context(tc.tile_pool(name="ps", bufs=8, space="PSUM"))

        # ---- Residents -------------------------------------------------
        # x-shard as 16 ki-pair tiles (1 MB DMAs, 4 KiB/descriptor), sync q.
        xres = []
        for kp in range(KC // 2):
            xk = xpool.tile([P, 2 * S_SH], f32r, tag="xres", name=f"x{kp}")
            xv = xk[:].rearrange("p (j n) -> p j n", j=2)
            if kp == 0:
                # Split so ki=0 (j=0 half) lands after 512 KiB, not 1 MiB:
                # the first matmul's critical path.
                nc.sync.dma_start(xv[:, 0:1, :], xt_r[kp][:, 0:1, :])
                nc.sync.dma_start(xv[:, 1:2, :], xt_r[kp][:, 1:2, :])
            else:
                nc.sync.dma_start(xv, xt_r[kp])
            xres.append(xk)

        def xs(ki, t):  # rhs slice for contraction chunk ki, token chunk t
            kp, j = divmod(ki, 2)
            off = j * S_SH + t * 512
            return xres[kp][:, off:off + 512]

        # Packed A (host layout [p, ki, lr]): small leading chunk so the
        # first LoRA matmul's operand lands in ~128 KiB, then 4 chunks.
        at_sb = cpool.tile([P, KC * LR], f32r, tag="at")
        nc.scalar.dma_start(at_sb[:, 0:2 * LR], at[:, 0:2 * LR])
        for g in range(4):
            lo = max(2 * LR, g * KC * LR // 4)
            hi = (g + 1) * KC * LR // 4
            nc.scalar.dma_start(at_sb[:, lo:hi], at[:, lo:hi])
        m1_sb = cpool.tile([P, S_SH], f32, tag="m1")
        nc.sync.dma_start(m1_sb[:], m1[:])
        bias_sb = cpool.tile([P, D_OUT // P], f32, tag="bias")
        nc.sync.dma_start(bias_sb[:], bias.rearrange("(j p) -> p j", p=P))
        btf_sb = cpool.tile([P, D_OUT], f32r, tag="btf")
        nc.sync.dma_start(btf_sb[:], btf[:])

        def finalize(po_mc_t, mb, mc, split_dma=False):
            """LoRA B-side accumulate + bias eviction + output DMA."""
            j = mb * MC + mc
            osb = opool.tile([P, S_SH], f32, tag="oevict", name=f"o{mb}_{mc}")
            for t in range(TC):
                nc.tensor.matmul(po_mc_t[t][:], btf_sb[:, j * P:(j + 1) * P],
                                 abar[t][:], start=False, stop=True)
                nc.scalar.activation(osb[:, t * 512:(t + 1) * 512],
                                     po_mc_t[t][:],
                                     mybir.ActivationFunctionType.Identity,
                                     bias=bias_sb[:, j:j + 1])
                if split_dma:
                    # Tail: spread final evictions across two queues.
                    eng = nc.gpsimd if t == 0 else nc.sync
                    eng.dma_start(
                        ot_r[j][:, t * 512:(t + 1) * 512],
                        osb[:, t * 512:(t + 1) * 512])
            if not split_dma:
                nc.gpsimd.dma_start(ot_r[j][:], osb[:])

        # ---- Startup ki loop: LoRA-A (2 banks) + mb0 mc0-2 (6 banks) ---
        pa = [psum.tile([P, 512], f32, tag="ps", name=f"pa{t}")
              for t in range(TC)]
        po0 = [[psum.tile([P, 512], f32, tag="ps", name=f"po0_{mc}_{t}")
                for t in range(TC)] for mc in range(3)]
        for ki in range(KC):
            wtile = wpool.tile([P, 512], f32r, tag="wstream", name=f"w0_{ki}")
            nc.scalar.dma_start(wtile[:], wt_r[ki][:, 0:512])
            for t in range(TC):
                nc.tensor.matmul(pa[t][:], at_sb[:, ki * P:(ki + 1) * P],
                                 xs(ki, t), start=(ki == 0), stop=(ki == KC - 1))
            for mc in range(3):
                for t in range(TC):
                    nc.tensor.matmul(po0[mc][t][:],
                                     wtile[:, mc * P:(mc + 1) * P],
                                     xs(ki, t), start=(ki == 0), stop=False)
        abar = []
        for t in range(TC):
            ab = apool.tile([P, 512], f32r, tag="abar", name=f"ab{t}")
            nc.vector.tensor_mul(ab[:], pa[t][:],
                                 m1_sb[:, t * 512:(t + 1) * 512])
            abar.append(ab)
        # ---- mb0c: deferred mc=3 of block 0 (host-packed W re-fetch) ---
        # Runs before the mc0-2 finalizes: po3 only needs the pa slots
        # (freed by the abar muls), so the PE refills right after startup.
        po3 = [psum.tile([P, 512], f32, tag="ps", name=f"po3_{t}")
               for t in range(TC)]
        for g in range(4):  # each chunk covers 8 contraction steps
            w3 = w0pool.tile([P, 8 * P], f32r, tag="w0c", name=f"w3_{g}")
            nc.scalar.dma_start(w3[:], wt3[:, g * 8 * P:(g + 1) * 8 * P])
            for kk in range(8):
                ki = g * 8 + kk
                for t in range(TC):
                    nc.tensor.matmul(po3[t][:], w3[:, kk * P:(kk + 1) * P],
                                     xs(ki, t), start=(ki == 0), stop=False)
        for mc in range(3):
            finalize(po0[mc], 0, mc)
        finalize(po3, 0, 3)

        # ---- Main blocks mb 1..7 ---------------------------------------
        for mb in range(1, MB):
            po = [[psum.tile([P, 512], f32, tag="ps", name=f"po{mb}_{mc}_{t}")
                   for t in range(TC)] for mc in range(MC)]
            for ki in range(KC):
                wtile = wpool.tile([P, 512], f32r, tag="wstream",
                                   name=f"w{mb}_{ki}")
                eng = nc.sync if ki % 2 == 0 else nc.scalar
                eng.dma_start(wtile[:], wt_r[ki][:, mb * 512:(mb + 1) * 512])
                for mc in range(MC):
                    for t in range(TC):
                        nc.tensor.matmul(po[mc][t][:],
                                         wtile[:, mc * P:(mc + 1) * P],
                                         xs(ki, t), start=(ki == 0), stop=False)
            for mc in range(MC):
                finalize(po[mc], mb, mc, split_dma=(mb == MB - 1))

    nc.compile()
    return nc


def get_nc():
    if "nc" not in _CACHE:
        _CACHE["nc"] = _build_nc()
    return _CACHE["nc"]


def prep_in_maps(x, weight, bias, A_buffer, B_buffer, weight_indices):
    """Host-side sharding + layout transforms + fp32r pre-rounding."""
    x = np.asarray(x, dtype=np.float32)
    weight = np.asarray(weight, dtype=np.float32)
    bias = np.asarray(bias, dtype=np.float32)
    A_buffer = np.asarray(A_buffer, dtype=np.float32)
    B_buffer = np.asarray(B_buffer, dtype=np.float32)
    weight_indices = np.asarray(weight_indices)

    wt = round_f32r(weight.T)                              # [D_IN, D_OUT]
    # wt3[p, ki*P + n] = wt[ki*P + p, 384 + n]: packed mc=3 cols of block 0
    wt3 = np.ascontiguousarray(
        wt.reshape(KC, P, D_OUT)[:, :, 384:512]
        .transpose(1, 0, 2).reshape(P, KC * P))
    # at[p, ki*LR + lr] = A_stack[lr, ki*P + p]  (PE-ready packed layout)
    a_stack_t = round_f32r(A_buffer.reshape(LR, D_IN).T)   # [D_IN, LR]
    at = np.ascontiguousarray(
        a_stack_t.reshape(KC, P, LR).transpose(1, 0, 2).reshape(P, KC * LR))
    # btf[l*R+r, m] = B_buffer[0, l, m, r] * SCALING
    btf = round_f32r(B_buffer[0].transpose(0, 2, 1).reshape(LR, D_OUT)
                     * SCALING)
    m1_full = np.repeat(
        (np.arange(L)[:, None] == weight_indices[None, :]), R, axis=0
    ).astype(np.float32)                                   # [LR, S]
    xt_full = round_f32r(x).T                              # view [D_IN, S]

    in_maps = []
    for c in range(N_CORES):
        sl = slice(c * S_SH, (c + 1) * S_SH)
        in_maps.append({
            "xt": np.ascontiguousarray(xt_full[:, sl]),
            "wt": wt,
            "wt3": wt3,
            "at": at,
            "btf": btf,
            "m1": np.ascontiguousarray(m1_full[:, sl]),
            "bias": bias,
        })
    return in_maps


def gather(results):
    out = np.empty((S, D_OUT), dtype=np.float32)
    for c in range(N_CORES):
        out[c * S_SH:(c + 1) * S_SH, :] = results[c]["ot"].T
    return out


def kernel(x, weight, bias, A_buffer, B_buffer, weight_indices):
    nc = get_nc()
    in_maps = prep_in_maps(x, weight, bias, A_buffer, B_buffer, weight_indices)
    res = run_bass_kernel_spmd(nc, in_maps, list(range(N_CORES)))
    return gather(res.results)
